# revision 2
# baseline (speedup 1.0000x reference)
"""Bellman/Kalman information filter (DFSV model) on 8 Trainium2 NeuronCores.

Structure exploited (all verified against the reference):
  - F and Q are block-diagonal and H = [Lambda, 0], so the information
    matrix stays exactly block-diagonal forever and the h-mean is never
    updated: h_t == mu for all t, and the h covariance block sits at its
    stationary Lyapunov solution, so Omega_h is constant.
  - The f-block covariance Riccati recursion is data-independent (Q_f =
    diag(exp(mu)) is constant) and converges (bitwise in fp32) by t=8.
  - Past convergence the mean recursion is linear time-invariant:
        x_t = A x_{t-1} + P_post * g_t,   A = (I - P_post J) Phi_f
    with spectral radius ~0.27, so a 16-tap truncated FIR reproduces the
    scan to < 1e-9 relative error.  Each core therefore computes its
    1024-step chunk independently from a 32-step warm-up window.
  - Log-lik terms via Woodbury: logdet S_t = sum(log sigma2) + ld(t) with
    ld(t) = logdet(I + J P_pred(t)) (data-independent, constant for t>=8)
    and v' S^-1 v = y'R^-1 y - f_pred'(g+u) - u' P_post u  (all K-dim).

Per-core device kernel does all O(T)-work: the Y pass (transpose + big
matmuls), the blocked-partition FIR scan (8 time-groups of 128 steps on
the partition axis, log2 doubling with block-diagonal A-power matmuls),
the innovation quadratics (accumulated into a single PSUM scalar), and
the means/infos tensor assembly + writeback (33 MB total).  Host
precomputes only the tiny (16x16, data-independent) Riccati / Lyapunov
constants and the 8-step exact prefix, shards the inputs, and
concatenates the outputs.
"""

import numpy as np

import concourse.bass as bass
import concourse.bacc as bacc
import concourse.mybir as mybir
from concourse import tile
from concourse.bass_utils import run_bass_kernel_spmd
from concourse.masks import make_identity

F32 = mybir.dt.float32
T, N, K = 8192, 128, 16
NCORES = 8
TC = T // NCORES          # 1024 steps per core
PRE = 32                  # warm-up rows prepended to each chunk
TW = TC + PRE             # 1056
NGRP = 8                  # time-groups per core (partition-blocked FIR)
GW = 144                  # group width: 16 warm-up cols + 128 steps
VO = 16                   # first valid column within a group
NE = 8                    # exact early steps (Riccati converged after)

_CACHED = {}


def _bd8(m):
    """Block-diagonal [128,128] with 8 copies of a 16x16 block."""
    out = np.zeros((128, 128), np.float64)
    for g in range(8):
        out[16 * g:16 * g + 16, 16 * g:16 * g + 16] = m
    return out


def _build_nc():
    nc = bacc.Bacc(None, target_bir_lowering=False)
    di = {}

    def din(name, shape):
        di[name] = nc.dram_tensor(name, list(shape), F32, kind="ExternalInput")

    din("y_chunk", (TW, N))
    din("lam", (N, K))
    din("sigma2", (N, 1))
    din("mu_row", (1, K))
    for k in range(4):
        din(f"bd_pow{k}", (128, 128))
    din("bd_phi", (128, 128))
    din("bd_negj", (128, 128))
    din("bd_pp", (128, 128))
    din("seq8", (K, NE))
    din("em8", (K, NE))
    din("oem8", (K, NE))
    din("e7", (K, NE))
    din("info_row", (1, 1024))
    din("info_early", (NE, 1024))
    din("consts", (1, 4))
    means_o = nc.dram_tensor("means_part", [TC, 2 * K], F32, kind="ExternalOutput")
    infos_o = nc.dram_tensor("infos_part", [TC, 1024], F32, kind="ExternalOutput")
    ll_o = nc.dram_tensor("ll_part", [1, 1], F32, kind="ExternalOutput")

    LOG2PI = float(np.log(2.0 * np.pi))

    with tile.TileContext(nc) as tc:
        from contextlib import ExitStack
        with ExitStack() as ctx:
            cst = ctx.enter_context(tc.tile_pool(name="cst", bufs=1))
            wrk = ctx.enter_context(tc.tile_pool(name="wrk", bufs=2))
            yin_pool = ctx.enter_context(tc.tile_pool(name="yinp", bufs=3))
            ps = ctx.enter_context(tc.tile_pool(name="ps", bufs=3, space="PSUM"))
            pscal = ctx.enter_context(tc.tile_pool(name="pscal", bufs=1, space="PSUM"))

            # ---------- constants / small inputs ----------
            ident = cst.tile([128, 128], F32)
            make_identity(nc, ident[:])
            ones_row = cst.tile([1, 128], F32)
            nc.gpsimd.memset(ones_row[:], 1.0)
            neg128 = cst.tile([128, 1], F32)
            nc.gpsimd.memset(neg128[:], -1.0)
            n1024 = cst.tile([128, 1], F32)
            nc.gpsimd.memset(n1024[:], float(TC))

            def load(name, shape):
                t_ = cst.tile(list(shape), F32, tag=name)
                nc.sync.dma_start(out=t_[:], in_=di[name][:])
                return t_

            lam_sb = load("lam", (N, K))
            sig_sb = load("sigma2", (N, 1))
            mu_sb = load("mu_row", (1, K))
            bdp = [load(f"bd_pow{k}", (128, 128)) for k in range(4)]
            bd_phi = load("bd_phi", (128, 128))
            bd_negj = load("bd_negj", (128, 128))
            bd_pp = load("bd_pp", (128, 128))
            seq8 = load("seq8", (K, NE))
            em8 = load("em8", (K, NE))
            oem8 = load("oem8", (K, NE))
            e7 = load("e7", (K, NE))
            irow = load("info_row", (1, 1024))
            iearly = load("info_early", (NE, 1024))
            consts = load("consts", (1, 4))

            # ---------- infos output (big DMAs, start early) ----------
            binfo = cst.tile([128, 1024], F32)
            for h in range(2):
                pb = ps.tile([128, 512], F32, tag="big")
                nc.tensor.matmul(pb[:], lhsT=ones_row[:],
                                 rhs=irow[:, 512 * h:512 * (h + 1)],
                                 start=True, stop=True)
                nc.scalar.copy(binfo[:, 512 * h:512 * (h + 1)], pb[:])
            nc.sync.dma_start(out=infos_o[0:NE, :], in_=iearly[:])
            for i in range(7):
                nc.sync.dma_start(out=infos_o[NE + 128 * i:NE + 128 * (i + 1), :],
                                  in_=binfo[:])
            nc.sync.dma_start(out=infos_o[NE + 128 * 7:TC, :],
                              in_=binfo[0:TC - NE - 128 * 7, :])

            # ---------- Y pass: load + transpose + G + ysq ----------
            yT = wrk.tile([128, TW], F32, tag="yT")
            nblk = TW // 128            # 8 full blocks
            for i in range(nblk):
                yt = yin_pool.tile([128, 128], F32, tag="yin")
                nc.sync.dma_start(out=yt[:], in_=di["y_chunk"][128 * i:128 * (i + 1), :])
                pt = ps.tile([128, 128], F32, tag="big")
                nc.tensor.transpose(pt[:], yt[:], ident[:])
                nc.scalar.copy(yT[:, 128 * i:128 * (i + 1)], pt[:])
            rem = TW - 128 * nblk       # 32
            yt = yin_pool.tile([128, 128], F32, tag="yin")
            nc.sync.dma_start(out=yt[0:rem, :], in_=di["y_chunk"][128 * nblk:TW, :])
            pt = ps.tile([128, 128], F32, tag="big")
            nc.tensor.transpose(pt[:, 0:rem], yt[0:rem, :], ident[0:rem, 0:rem])
            nc.scalar.copy(yT[:, 128 * nblk:TW], pt[:, 0:rem])

            isig = cst.tile([128, 1], F32)
            nc.vector.reciprocal(isig[:], sig_sb[:])
            logsig = cst.tile([128, 1], F32)
            nc.scalar.activation(logsig[:], sig_sb[:], mybir.ActivationFunctionType.Ln)
            ltri = cst.tile([N, K], F32)
            nc.vector.tensor_scalar_mul(ltri[:], lam_sb[:], isig[:])

            ysq = wrk.tile([128, TW], F32, tag="ysq")
            nc.scalar.activation(ysq[:], yT[:], mybir.ActivationFunctionType.Square)
            rysq = wrk.tile([128, 1], F32, tag="rysq")
            nc.vector.tensor_reduce(rysq[:], ysq[:, PRE:TW], axis=mybir.AxisListType.X,
                                    op=mybir.AluOpType.add)

            g_sb = wrk.tile([K, TW], F32, tag="g")
            for c0, cw in ((0, 512), (512, 512), (1024, TW - 1024)):
                pg = ps.tile([K, 512], F32, tag="gband")
                nc.tensor.matmul(pg[:, 0:cw], lhsT=ltri[:], rhs=yT[:, c0:c0 + cw],
                                 start=True, stop=True)
                nc.scalar.copy(g_sb[:, c0:c0 + cw], pg[:, 0:cw])

            # ---------- blocked layout: 8 time-groups on partitions ----------
            gblk = wrk.tile([128, GW], F32, tag="gblk")
            for g in range(NGRP):
                og = PRE + 128 * g - VO
                nc.sync.dma_start(out=gblk[16 * g:16 * (g + 1), :],
                                  in_=g_sb[:, og:og + GW])

            pbb = ps.tile([128, GW], F32, tag="big")
            nc.tensor.matmul(pbb[:], lhsT=bd_pp[:], rhs=gblk[:], start=True, stop=True)
            bblk = wrk.tile([128, GW], F32, tag="xfir")
            nc.scalar.copy(bblk[:], pbb[:])

            # c-surgery: zero the pre-convergence prefix, inject exact x_7
            # (data-driven; masks make it a no-op on cores > 0)
            t1 = wrk.tile([K, NE], F32, tag="t1")
            t2 = wrk.tile([K, NE], F32, tag="t2")
            nc.vector.tensor_mul(t1[:], bblk[0:16, VO:VO + NE], oem8[:])
            nc.vector.tensor_mul(t2[:], seq8[:], e7[:])
            nc.vector.tensor_add(bblk[0:16, VO:VO + NE], t1[:], t2[:])

            # ---------- FIR scan via log2 doubling ----------
            xk = bblk
            for k in range(4):
                s = 1 << k
                px = ps.tile([128, GW], F32, tag="big")
                nc.tensor.matmul(px[:], lhsT=ident[:], rhs=xk[:], start=True, stop=False)
                nc.tensor.matmul(px[:, s:GW], lhsT=bdp[k][:], rhs=xk[:, 0:GW - s],
                                 start=False, stop=True)
                xk2 = wrk.tile([128, GW], F32, tag="xfir")
                nc.scalar.copy(xk2[:], px[:])
                xk = xk2

            # blend exact early means into group 0 (no-op on cores > 0)
            b1 = wrk.tile([K, NE], F32, tag="t1")
            b2 = wrk.tile([K, NE], F32, tag="t2")
            nc.vector.tensor_mul(b1[:], xk[0:16, VO:VO + NE], oem8[:])
            nc.vector.tensor_mul(b2[:], seq8[:], em8[:])
            nc.vector.tensor_add(xk[0:16, VO:VO + NE], b1[:], b2[:])

            # ---------- F_pred / U / W ----------
            pf = ps.tile([128, GW], F32, tag="big")
            nc.tensor.matmul(pf[:, 1:GW], lhsT=bd_phi[:], rhs=xk[:, 0:GW - 1],
                             start=True, stop=True)
            fp = wrk.tile([128, GW], F32, tag="fp")
            nc.scalar.copy(fp[:, 1:GW], pf[:, 1:GW])

            pu = ps.tile([128, GW], F32, tag="big")
            nc.tensor.matmul(pu[:], lhsT=ident[:], rhs=gblk[:], start=True, stop=False)
            nc.tensor.matmul(pu[:, VO:GW], lhsT=bd_negj[:], rhs=fp[:, VO:GW],
                             start=False, stop=True)
            ub = wrk.tile([128, GW], F32, tag="ub")
            nc.scalar.copy(ub[:], pu[:])

            pw = ps.tile([128, GW], F32, tag="big")
            nc.tensor.matmul(pw[:, VO:GW], lhsT=bd_pp[:], rhs=ub[:, VO:GW],
                             start=True, stop=True)
            wb = wrk.tile([128, 128], F32, tag="wb")
            nc.scalar.copy(wb[:], pw[:, VO:GW])

            s1 = wrk.tile([128, 128], F32, tag="s1")
            nc.vector.tensor_add(s1[:], gblk[:, VO:GW], ub[:, VO:GW])
            prod1 = wrk.tile([128, 128], F32, tag="prod1")
            nc.vector.tensor_mul(prod1[:], fp[:, VO:GW], s1[:])
            prod2 = wrk.tile([128, 128], F32, tag="prod2")
            nc.vector.tensor_mul(prod2[:], ub[:, VO:GW], wb[:])
            r1 = wrk.tile([128, 1], F32, tag="r1")
            nc.vector.tensor_reduce(r1[:], prod1[:], axis=mybir.AxisListType.X,
                                    op=mybir.AluOpType.add)
            r2 = wrk.tile([128, 1], F32, tag="r2")
            nc.vector.tensor_reduce(r2[:], prod2[:], axis=mybir.AxisListType.X,
                                    op=mybir.AluOpType.add)

            # ---------- log-lik scalar ----------
            pscal_t = pscal.tile([1, 1], F32)
            nc.tensor.matmul(pscal_t[:], lhsT=isig[:], rhs=rysq[:], start=True, stop=False)
            nc.tensor.matmul(pscal_t[:], lhsT=neg128[:], rhs=r1[:], start=False, stop=False)
            nc.tensor.matmul(pscal_t[:], lhsT=neg128[:], rhs=r2[:], start=False, stop=False)
            nc.tensor.matmul(pscal_t[:], lhsT=n1024[:], rhs=logsig[:], start=False, stop=True)
            lt1 = wrk.tile([1, 1], F32, tag="lt1")
            nc.vector.tensor_add(lt1[:], pscal_t[:], consts[:, 0:1])
            lt2 = wrk.tile([1, 1], F32, tag="lt2")
            nc.vector.tensor_add(lt2[:], lt1[:], consts[:, 1:2])
            ll_sb = wrk.tile([1, 1], F32, tag="llsb")
            nc.scalar.activation(ll_sb[:], lt2[:], mybir.ActivationFunctionType.Copy,
                                 scale=-0.5, bias=-0.5 * TC * N * LOG2PI)
            nc.sync.dma_start(out=ll_o[:], in_=ll_sb[:])

            # ---------- means output ----------
            pxt = ps.tile([128, 128], F32, tag="big")
            nc.tensor.transpose(pxt[:], xk[:, VO:VO + 128], ident[:])
            xt = wrk.tile([128, 128], F32, tag="xt")
            nc.scalar.copy(xt[:], pxt[:])
            pmu = ps.tile([128, 32], F32, tag="big")
            nc.tensor.matmul(pmu[:, 0:16], lhsT=ones_row[:], rhs=mu_sb[:],
                             start=True, stop=True)
            mub = wrk.tile([128, 16], F32, tag="mub")
            nc.scalar.copy(mub[:], pmu[:, 0:16])
            for g in range(NGRP):
                mt = wrk.tile([128, 32], F32, tag="mt")
                nc.vector.tensor_copy(mt[:, 0:16], xt[:, 16 * g:16 * (g + 1)])
                nc.vector.tensor_copy(mt[:, 16:32], mub[:])
                nc.sync.dma_start(out=means_o[128 * g:128 * (g + 1), :], in_=mt[:])

    nc.finalize()
    return nc


def _host_constants(lam, Phi_f, Phi_h, mu, sigma2, Q_h, Y):
    """Tiny data-independent constants (float64) + the 8-step exact prefix."""
    lam = lam.astype(np.float64)
    Phi_f = Phi_f.astype(np.float64)
    Phi_h = Phi_h.astype(np.float64)
    mu = mu.astype(np.float64)
    sigma2 = sigma2.astype(np.float64)
    Q_h = Q_h.astype(np.float64)
    I = np.eye(K)

    LtRi = lam.T / sigma2
    J = LtRi @ lam
    Qf = np.diag(np.exp(mu))

    # h block: stationary Lyapunov solution
    P_h = np.linalg.solve(np.eye(K * K) - np.kron(Phi_h, Phi_h), Q_h.reshape(-1)).reshape(K, K)
    P_h = 0.5 * (P_h + P_h.T)
    Om_h = np.linalg.inv(P_h)

    # f block Riccati (data independent); converged long before t=16
    P_post = I.copy()
    Ppost_l, Om_l, ld_l, A_l = [], [], [], []
    for _ in range(16):
        P_pred = Phi_f @ P_post @ Phi_f.T + Qf
        Om_pred = np.linalg.inv(P_pred)
        Om_post = Om_pred + J
        P_post = np.linalg.inv(Om_post)
        P_post = 0.5 * (P_post + P_post.T)
        Ppost_l.append(P_post)
        Om_l.append(Om_post)
        ld_l.append(np.linalg.slogdet(I + J @ P_pred)[1])
        A_l.append((I - P_post @ J) @ Phi_f)
    P_ss = Ppost_l[-1]
    A_ss = A_l[-1]
    Om_ss = Om_l[-1]
    ld_ss = ld_l[-1]

    pows = [A_ss]
    for _ in range(3):
        pows.append(pows[-1] @ pows[-1])

    # exact 8-step prefix on core 0's data
    x = np.zeros(K)
    seq8 = np.zeros((K, NE))
    corr = 0.0
    for t_ in range(NE):
        fpred = Phi_f @ x
        g = LtRi @ Y[t_].astype(np.float64)
        u = g - J @ fpred
        x = fpred + Ppost_l[t_] @ u
        seq8[:, t_] = x
        corr += u @ ((P_ss - Ppost_l[t_]) @ u)

    def info_flat(om_f):
        row = np.zeros(1024)
        for k_ in range(K):
            row[32 * k_:32 * k_ + K] = om_f[k_]
            row[32 * (K + k_) + K:32 * (K + k_) + 2 * K] = Om_h[k_]
        return row

    info_row = info_flat(Om_ss)
    info_early = np.stack([info_flat(Om_l[t_]) for t_ in range(NE)])

    ld_early = float(np.sum(ld_l[:NE]))

    return dict(
        J=J, P_ss=P_ss, A_pows=pows, Phi_f=Phi_f,
        seq8=seq8, corr=corr, info_row=info_row, info_early=info_early,
        ld_early=ld_early, ld_ss=ld_ss,
    )


def kernel(observations, lambda_r, Phi_f, Phi_h, mu, sigma2, Q_h, _trace=False):
    Y = np.ascontiguousarray(observations, np.float32)
    hc = _host_constants(lambda_r, Phi_f, Phi_h, mu, sigma2, Q_h, Y)

    f32 = lambda a: np.ascontiguousarray(a, np.float32)
    base = {
        "lam": f32(np.asarray(lambda_r).reshape(N, K)),
        "sigma2": f32(np.asarray(sigma2).reshape(N, 1)),
        "mu_row": f32(np.asarray(mu).reshape(1, K)),
        "bd_phi": f32(_bd8(hc["Phi_f"].T)),
        "bd_negj": f32(_bd8(-hc["J"])),
        "bd_pp": f32(_bd8(hc["P_ss"])),
        "info_row": f32(hc["info_row"].reshape(1, 1024)),
    }
    for k in range(4):
        base[f"bd_pow{k}"] = f32(_bd8(hc["A_pows"][k].T))

    in_maps = []
    for c in range(NCORES):
        t0 = c * TC
        yc = np.zeros((TW, N), np.float32)
        if c > 0:
            yc[0:PRE, :] = Y[t0 - PRE:t0]
        yc[PRE:, :] = Y[t0:t0 + TC]
        m = dict(base)
        m["y_chunk"] = yc
        if c == 0:
            m["seq8"] = f32(hc["seq8"])
            m["em8"] = np.ones((K, NE), np.float32)
            m["oem8"] = np.zeros((K, NE), np.float32)
            e7 = np.zeros((K, NE), np.float32)
            e7[:, NE - 1] = 1.0
            m["e7"] = e7
            m["info_early"] = f32(hc["info_early"])
            ld_tot = hc["ld_early"] + (TC - NE) * hc["ld_ss"]
            m["consts"] = f32(np.array([[ld_tot, hc["corr"], 0.0, 0.0]]))
        else:
            m["seq8"] = np.zeros((K, NE), np.float32)
            m["em8"] = np.zeros((K, NE), np.float32)
            m["oem8"] = np.ones((K, NE), np.float32)
            m["e7"] = np.zeros((K, NE), np.float32)
            m["info_early"] = f32(np.tile(hc["info_row"], (NE, 1)))
            m["consts"] = f32(np.array([[TC * hc["ld_ss"], 0.0, 0.0, 0.0]]))
        in_maps.append(m)

    if "nc" not in _CACHED:
        _CACHED["nc"] = _build_nc()
    res = run_bass_kernel_spmd(_CACHED["nc"], in_maps, list(range(NCORES)),
                               trace=_trace)
    _CACHED["last_result"] = res

    means = np.concatenate([res.results[c]["means_part"] for c in range(NCORES)], axis=0)
    infos = np.concatenate([res.results[c]["infos_part"] for c in range(NCORES)],
                           axis=0).reshape(T, 2 * K, 2 * K)
    ll = np.float32(np.sum([res.results[c]["ll_part"][0, 0] for c in range(NCORES)],
                           dtype=np.float64))
    return means, infos, ll


# revision 12
# speedup vs baseline: 1.2895x; 1.2895x over previous
"""Bellman/Kalman information filter (DFSV model) on 8 Trainium2 NeuronCores.

Structure exploited (all verified against the reference):
  - F and Q are block-diagonal and H = [Lambda, 0], so the information
    matrix stays exactly block-diagonal forever and the h-mean is never
    updated: h_t == mu for all t, and the h covariance block sits at its
    stationary Lyapunov solution, so Omega_h is constant.
  - The f-block covariance Riccati recursion is data-independent (Q_f =
    diag(exp(mu)) is constant) and converges (bitwise in fp32) by t=8.
  - Past convergence the mean recursion is linear time-invariant:
        x_t = A x_{t-1} + P_post * g_t,   A = (I - P_post J) Phi_f
    with spectral radius ~0.27, so a 16-tap truncated FIR reproduces the
    scan to < 1e-9 relative error.  Each core therefore computes its
    1024-step chunk independently from a 32-step warm-up window.
  - Log-lik terms via Woodbury: logdet S_t = sum(log sigma2) + ld(t) with
    ld(t) = logdet(I + J P_pred(t)) (data-independent, constant for t>=8)
    and v' S^-1 v = y'R^-1 y - f_pred'(g+u) - u' P_post u  (all K-dim).

Per-core device kernel does all O(T)-work: the Y pass (transpose + big
matmuls), the blocked-partition FIR scan (8 time-groups of 128 steps on
the partition axis, log2 doubling with block-diagonal A-power matmuls),
the innovation quadratics (accumulated into a single PSUM scalar), and
the means/infos tensor assembly + writeback (33 MB total).  Host
precomputes only the tiny (16x16, data-independent) Riccati / Lyapunov
constants and the 8-step exact prefix, shards the inputs, and
concatenates the outputs.
"""

import numpy as np

import concourse.bass as bass
import concourse.bacc as bacc
import concourse.mybir as mybir
from concourse import tile
from concourse.bass_utils import run_bass_kernel_spmd
from concourse.masks import make_identity

F32 = mybir.dt.float32
T, N, K = 8192, 128, 16
NCORES = 8
TC = T // NCORES          # 1024 steps per core
PRE = 32                  # warm-up rows prepended to each chunk
TW = TC + PRE             # 1056
NGRP = 8                  # time-groups per core (partition-blocked FIR)
GW = 144                  # group width: 16 warm-up cols + 128 steps
VO = 16                   # first valid column within a group
NE = 8                    # exact early steps (Riccati converged after)

_CACHED = {}


def _bd8(m):
    """Block-diagonal [128,128] with 8 copies of a 16x16 block."""
    out = np.zeros((128, 128), np.float64)
    for g in range(8):
        out[16 * g:16 * g + 16, 16 * g:16 * g + 16] = m
    return out


def _build_nc():
    nc = bacc.Bacc(None, target_bir_lowering=False)

    y_in = nc.dram_tensor("y_chunk", [TW, N], F32, kind="ExternalInput")
    # bdpack columns: [0:128] Phi', [128:256] J, [256:384] P_ss,
    #                 [384:896] (A^1)', (A^2)', (A^4)', (A^8)'
    bdpack = nc.dram_tensor("bdpack", [128, 896], F32, kind="ExternalInput")
    # spack columns: [0:16] lambda, [16] sigma2, [17] warm-up row mask
    spack = nc.dram_tensor("spack", [128, 18], F32, kind="ExternalInput")
    # kpack columns: [0:8] seq8, [8:16] em8, [16:24] oem8, [24:32] e7
    kpack = nc.dram_tensor("kpack", [K, 32], F32, kind="ExternalInput")
    # rpack (single row, base partition 0): [0:1024] info_row,
    #   [1024:1152] sigma2 as a row, [1152:1168] mu, [1168:1172] consts
    rpack = nc.dram_tensor("rpack", [1, 1172], F32, kind="ExternalInput")
    # epack: the 8 early info rows (DMA'd straight to the output)
    epack = nc.dram_tensor("epack", [NE, 1024], F32, kind="ExternalInput")
    means_o = nc.dram_tensor("means_part", [TC, 2 * K], F32, kind="ExternalOutput")
    infos_o = nc.dram_tensor("infos_part", [TC, 1024], F32, kind="ExternalOutput")
    ll_o = nc.dram_tensor("ll_part", [1, 1], F32, kind="ExternalOutput")

    LOG2PI = float(np.log(2.0 * np.pi))

    with tile.TileContext(nc) as tc:
        from contextlib import ExitStack
        with ExitStack() as ctx:
            cst = ctx.enter_context(tc.tile_pool(name="cst", bufs=1))
            wrk = ctx.enter_context(tc.tile_pool(name="wrk", bufs=2))
            yin_pool = ctx.enter_context(tc.tile_pool(name="yinp", bufs=4))
            ps = ctx.enter_context(tc.tile_pool(name="ps", bufs=3, space="PSUM"))
            psg = ctx.enter_context(tc.tile_pool(name="psg", bufs=3, space="PSUM"))
            pscal = ctx.enter_context(tc.tile_pool(name="pscal", bufs=1, space="PSUM"))

            # ---------- input DMAs (y on sync queue, packs on scalar queue) ----
            ytiles = []
            nblk = TW // 128            # 8 full blocks + 32-row remainder
            for i in range(nblk):
                yt = yin_pool.tile([128, 128], F32, tag="yin")
                nc.sync.dma_start(out=yt[:], in_=y_in[128 * i:128 * (i + 1), :])
                ytiles.append(yt)
            rem = TW - 128 * nblk       # 32
            ytr = yin_pool.tile([128, 128], F32, tag="yin")
            nc.sync.dma_start(out=ytr[0:rem, :], in_=y_in[128 * nblk:TW, :])
            ytiles.append(ytr)

            bd_sb = cst.tile([128, 896], F32)
            nc.scalar.dma_start(out=bd_sb[:], in_=bdpack[:])
            bd_phi = bd_sb[:, 0:128]
            bd_j = bd_sb[:, 128:256]
            bd_pp = bd_sb[:, 256:384]
            bdp = [bd_sb[:, 384 + 128 * k:512 + 128 * k] for k in range(4)]
            sp_sb = cst.tile([128, 18], F32)
            nc.scalar.dma_start(out=sp_sb[:], in_=spack[:])
            lam_sb = sp_sb[:, 0:16]
            sig_sb = sp_sb[:, 16:17]
            mask0 = sp_sb[:, 17:18]
            kp_sb = cst.tile([K, 32], F32)
            nc.scalar.dma_start(out=kp_sb[:], in_=kpack[:])
            seq8, em8, oem8, e7 = (kp_sb[:, 8 * j:8 * (j + 1)] for j in range(4))
            rp_sb = cst.tile([1, 1172], F32)
            nc.scalar.dma_start(out=rp_sb[:], in_=rpack[:])
            ep_sb = cst.tile([NE, 1024], F32)
            nc.scalar.dma_start(out=ep_sb[:], in_=epack[:])
            irow = rp_sb[:, 0:1024]
            sig_row = rp_sb[:, 1024:1152]
            mu_sb = rp_sb[:, 1152:1168]
            consts = rp_sb[:, 1168:1172]

            # ---------- tiny constants ----------
            ident = cst.tile([128, 128], F32)
            make_identity(nc, ident[:])
            ones_row = cst.tile([1, 128], F32)
            nc.gpsimd.memset(ones_row[:], 1.0)
            neg128 = cst.tile([128, 1], F32)
            nc.gpsimd.memset(neg128[:], -1.0)
            n1024 = cst.tile([128, 1], F32)
            nc.gpsimd.memset(n1024[:], float(TC))
            ones_p = cst.tile([128, 1], F32)
            nc.gpsimd.memset(ones_p[:], 1.0)

            isig = cst.tile([128, 1], F32)
            nc.vector.reciprocal(isig[:], sig_sb)
            logsig = cst.tile([128, 1], F32)
            nc.scalar.activation(logsig[:], sig_sb, mybir.ActivationFunctionType.Ln)
            isig_row = cst.tile([1, 128], F32)
            nc.vector.reciprocal(isig_row[:], sig_row)
            ltri = cst.tile([N, K], F32)
            nc.vector.tensor_scalar_mul(ltri[:], lam_sb, isig[:])
            # iw[p, n] = 1/sigma2[n]  (for the natural-layout y'R^-1 y pass)
            piw = ps.tile([128, 128], F32, tag="big")
            nc.tensor.matmul(piw[:], lhsT=ones_row[:], rhs=isig_row[:],
                             start=True, stop=True)
            iw = cst.tile([128, 128], F32)
            nc.scalar.copy(iw[:], piw[:])

            # ---------- per-block: transpose -> G block; square -> q ----------
            yT = wrk.tile([128, TW], F32, tag="yT")
            g_sb = wrk.tile([K, TW], F32, tag="g")
            qacc = cst.tile([128, 1], F32)
            nc.gpsimd.memset(qacc[:], 0.0)
            for i in range(nblk + 1):
                yt = ytiles[i]
                cw = 128 if i < nblk else rem
                pt = ps.tile([128, 128], F32, tag="big")
                nc.tensor.transpose(pt[:, 0:cw], yt[0:cw, :], ident[0:cw, 0:cw])
                nc.scalar.copy(yT[:, 128 * i:128 * i + cw], pt[:, 0:cw])
                pgb = psg.tile([K, 128], F32, tag="gband")
                nc.tensor.matmul(pgb[:, 0:cw], lhsT=ltri[:],
                                 rhs=yT[:, 128 * i:128 * i + cw],
                                 start=True, stop=True)
                nc.scalar.copy(g_sb[:, 128 * i:128 * i + cw], pgb[:, 0:cw])
                # natural-layout weighted square -> q accumulation
                p1 = rem if i == nblk else 128
                sq = wrk.tile([128, 128], F32, tag="sq")
                nc.scalar.activation(sq[0:p1, :], yt[0:p1, :],
                                     mybir.ActivationFunctionType.Square)
                nc.vector.tensor_mul(sq[0:p1, :], sq[0:p1, :], iw[0:p1, :])
                qb = wrk.tile([128, 1], F32, tag="qb")
                nc.vector.tensor_reduce(qb[0:p1, :], sq[0:p1, :],
                                        axis=mybir.AxisListType.X,
                                        op=mybir.AluOpType.add)
                if i == 0:
                    nc.vector.tensor_mul(qb[:], qb[:], mask0[:])
                nc.vector.tensor_add(qacc[0:p1, :], qacc[0:p1, :], qb[0:p1, :])

            # ---------- infos output (overlaps the FIR chain) ----------
            binfo = cst.tile([128, 1024], F32)
            for h in range(2):
                pb = ps.tile([128, 512], F32, tag="big")
                nc.tensor.matmul(pb[:], lhsT=ones_row[:],
                                 rhs=irow[:, 512 * h:512 * (h + 1)],
                                 start=True, stop=True)
                nc.scalar.copy(binfo[:, 512 * h:512 * (h + 1)], pb[:])
            nc.sync.dma_start(out=infos_o[0:NE, :], in_=ep_sb[:])
            for i in range(7):
                nc.sync.dma_start(out=infos_o[NE + 128 * i:NE + 128 * (i + 1), :],
                                  in_=binfo[:])
            nc.sync.dma_start(out=infos_o[NE + 128 * 7:TC, :],
                              in_=binfo[0:TC - NE - 128 * 7, :])

            # ---------- blocked layout: 8 time-groups on partitions ----------
            gblk = wrk.tile([128, GW], F32, tag="gblk")
            for g in range(NGRP):
                og = PRE + 128 * g - VO
                eng = nc.scalar if g % 2 else nc.sync
                eng.dma_start(out=gblk[16 * g:16 * (g + 1), :],
                              in_=g_sb[:, og:og + GW])

            pbb = ps.tile([128, GW], F32, tag="big")
            nc.tensor.matmul(pbb[:], lhsT=bd_pp, rhs=gblk[:], start=True, stop=True)
            bblk = wrk.tile([128, GW], F32, tag="xfir")
            nc.scalar.copy(bblk[:], pbb[:])

            # c-surgery: zero the pre-convergence prefix, inject exact x_7
            # (data-driven; masks make it a no-op on cores > 0)
            t1 = wrk.tile([K, NE], F32, tag="t1")
            t2 = wrk.tile([K, NE], F32, tag="t2")
            nc.vector.tensor_mul(t1[:], bblk[0:16, VO:VO + NE], oem8)
            nc.vector.tensor_mul(t2[:], seq8, e7)
            nc.vector.tensor_add(bblk[0:16, VO:VO + NE], t1[:], t2[:])

            # ---------- FIR scan via log2 doubling ----------
            xk = bblk
            for k in range(4):
                s = 1 << k
                px = ps.tile([128, GW], F32, tag="big")
                nc.tensor.matmul(px[:, s:GW], lhsT=bdp[k], rhs=xk[:, 0:GW - s],
                                 start=True, stop=True)
                xk2 = wrk.tile([128, GW], F32, tag="xfir")
                nc.vector.tensor_copy(xk2[:, 0:s], xk[:, 0:s])
                nc.vector.tensor_add(xk2[:, s:GW], px[:, s:GW], xk[:, s:GW])
                xk = xk2

            # blend exact early means into group 0 (no-op on cores > 0)
            b1 = wrk.tile([K, NE], F32, tag="t1")
            b2 = wrk.tile([K, NE], F32, tag="t2")
            nc.vector.tensor_mul(b1[:], xk[0:16, VO:VO + NE], oem8)
            nc.vector.tensor_mul(b2[:], seq8, em8)
            nc.vector.tensor_add(xk[0:16, VO:VO + NE], b1[:], b2[:])

            # ---------- F_pred / U / W ----------
            pf = ps.tile([128, GW], F32, tag="big")
            nc.tensor.matmul(pf[:, VO:GW], lhsT=bd_phi, rhs=xk[:, VO - 1:GW - 1],
                             start=True, stop=True)
            fp = wrk.tile([128, GW], F32, tag="fp")
            nc.scalar.copy(fp[:, VO:GW], pf[:, VO:GW])

            pu = ps.tile([128, GW], F32, tag="big")
            nc.tensor.matmul(pu[:, VO:GW], lhsT=bd_j, rhs=fp[:, VO:GW],
                             start=True, stop=True)
            ub = wrk.tile([128, GW], F32, tag="ub")
            nc.vector.tensor_sub(ub[:, VO:GW], gblk[:, VO:GW], pu[:, VO:GW])

            pw = ps.tile([128, GW], F32, tag="big")
            nc.tensor.matmul(pw[:, VO:GW], lhsT=bd_pp, rhs=ub[:, VO:GW],
                             start=True, stop=True)
            wb = wrk.tile([128, 128], F32, tag="wb")
            nc.scalar.copy(wb[:], pw[:, VO:GW])

            s1 = wrk.tile([128, 128], F32, tag="s1")
            nc.vector.tensor_add(s1[:], gblk[:, VO:GW], ub[:, VO:GW])
            prod1 = wrk.tile([128, 128], F32, tag="prod1")
            nc.vector.tensor_mul(prod1[:], fp[:, VO:GW], s1[:])
            prod2 = wrk.tile([128, 128], F32, tag="prod2")
            nc.vector.tensor_mul(prod2[:], ub[:, VO:GW], wb[:])
            r1 = wrk.tile([128, 1], F32, tag="r1")
            nc.vector.tensor_reduce(r1[:], prod1[:], axis=mybir.AxisListType.X,
                                    op=mybir.AluOpType.add)
            r2 = wrk.tile([128, 1], F32, tag="r2")
            nc.vector.tensor_reduce(r2[:], prod2[:], axis=mybir.AxisListType.X,
                                    op=mybir.AluOpType.add)

            # ---------- log-lik scalar ----------
            pscal_t = pscal.tile([1, 1], F32)
            nc.tensor.matmul(pscal_t[:], lhsT=ones_p[:], rhs=qacc[:], start=True, stop=False)
            nc.tensor.matmul(pscal_t[:], lhsT=neg128[:], rhs=r1[:], start=False, stop=False)
            nc.tensor.matmul(pscal_t[:], lhsT=neg128[:], rhs=r2[:], start=False, stop=False)
            nc.tensor.matmul(pscal_t[:], lhsT=n1024[:], rhs=logsig[:], start=False, stop=True)
            lt1 = wrk.tile([1, 1], F32, tag="lt1")
            nc.vector.tensor_add(lt1[:], pscal_t[:], consts[:, 0:1])
            lt2 = wrk.tile([1, 1], F32, tag="lt2")
            nc.vector.tensor_add(lt2[:], lt1[:], consts[:, 1:2])
            ll_sb = wrk.tile([1, 1], F32, tag="llsb")
            nc.scalar.activation(ll_sb[:], lt2[:], mybir.ActivationFunctionType.Copy,
                                 scale=-0.5, bias=-0.5 * TC * N * LOG2PI)
            nc.sync.dma_start(out=ll_o[:], in_=ll_sb[:])

            # ---------- means output ----------
            pxt = ps.tile([128, 128], F32, tag="big")
            nc.tensor.transpose(pxt[:], xk[:, VO:VO + 128], ident[:])
            xt = wrk.tile([128, 128], F32, tag="xt")
            nc.scalar.copy(xt[:], pxt[:])
            pmu = ps.tile([128, 32], F32, tag="big")
            nc.tensor.matmul(pmu[:, 0:16], lhsT=ones_row[:], rhs=mu_sb,
                             start=True, stop=True)
            mub = wrk.tile([128, 16], F32, tag="mub")
            nc.scalar.copy(mub[:], pmu[:, 0:16])
            for g in range(NGRP):
                mt = wrk.tile([128, 32], F32, tag="mt")
                nc.vector.tensor_copy(mt[:, 0:16], xt[:, 16 * g:16 * (g + 1)])
                nc.vector.tensor_copy(mt[:, 16:32], mub[:])
                eng = nc.scalar if g % 2 else nc.sync
                eng.dma_start(out=means_o[128 * g:128 * (g + 1), :], in_=mt[:])

    nc.finalize()
    return nc


def _host_constants(lam, Phi_f, Phi_h, mu, sigma2, Q_h, Y):
    """Tiny data-independent constants (float64) + the 8-step exact prefix."""
    lam = lam.astype(np.float64)
    Phi_f = Phi_f.astype(np.float64)
    Phi_h = Phi_h.astype(np.float64)
    mu = mu.astype(np.float64)
    sigma2 = sigma2.astype(np.float64)
    Q_h = Q_h.astype(np.float64)
    I = np.eye(K)

    LtRi = lam.T / sigma2
    J = LtRi @ lam
    Qf = np.diag(np.exp(mu))

    # h block: stationary Lyapunov solution
    P_h = np.linalg.solve(np.eye(K * K) - np.kron(Phi_h, Phi_h), Q_h.reshape(-1)).reshape(K, K)
    P_h = 0.5 * (P_h + P_h.T)
    Om_h = np.linalg.inv(P_h)

    # f block Riccati (data independent); converged long before t=16
    P_post = I.copy()
    Ppost_l, Om_l, ld_l = [], [], []
    for _ in range(16):
        P_pred = Phi_f @ P_post @ Phi_f.T + Qf
        Om_pred = np.linalg.inv(P_pred)
        Om_post = Om_pred + J
        P_post = np.linalg.inv(Om_post)
        P_post = 0.5 * (P_post + P_post.T)
        Ppost_l.append(P_post)
        Om_l.append(Om_post)
        ld_l.append(np.linalg.slogdet(I + J @ P_pred)[1])
    P_ss = Ppost_l[-1]
    A_ss = (I - P_ss @ J) @ Phi_f
    Om_ss = Om_l[-1]
    ld_ss = ld_l[-1]

    pows = [A_ss]
    for _ in range(3):
        pows.append(pows[-1] @ pows[-1])

    # exact 8-step prefix on core 0's data
    x = np.zeros(K)
    seq8 = np.zeros((K, NE))
    corr = 0.0
    for t_ in range(NE):
        fpred = Phi_f @ x
        g = LtRi @ Y[t_].astype(np.float64)
        u = g - J @ fpred
        x = fpred + Ppost_l[t_] @ u
        seq8[:, t_] = x
        corr += u @ ((P_ss - Ppost_l[t_]) @ u)

    def info_flat(om_f):
        row = np.zeros(1024)
        for k_ in range(K):
            row[32 * k_:32 * k_ + K] = om_f[k_]
            row[32 * (K + k_) + K:32 * (K + k_) + 2 * K] = Om_h[k_]
        return row

    info_row = info_flat(Om_ss)
    info_early = np.stack([info_flat(Om_l[t_]) for t_ in range(NE)])

    return dict(
        J=J, P_ss=P_ss, A_pows=pows, Phi_f=Phi_f,
        seq8=seq8, corr=corr, info_row=info_row, info_early=info_early,
        ld_early=float(np.sum(ld_l[:NE])), ld_ss=ld_ss,
    )


def kernel(observations, lambda_r, Phi_f, Phi_h, mu, sigma2, Q_h, _trace=False):
    Y = np.ascontiguousarray(observations, np.float32)
    hc = _host_constants(lambda_r, Phi_f, Phi_h, mu, sigma2, Q_h, Y)

    f32 = lambda a: np.ascontiguousarray(a, np.float32)

    bdpack = np.zeros((128, 896), np.float64)
    bdpack[:, 0:128] = _bd8(hc["Phi_f"].T)
    bdpack[:, 128:256] = _bd8(hc["J"])
    bdpack[:, 256:384] = _bd8(hc["P_ss"])
    for k in range(4):
        bdpack[:, 384 + 128 * k:512 + 128 * k] = _bd8(hc["A_pows"][k].T)

    spack = np.zeros((128, 18), np.float64)
    spack[:, 0:16] = np.asarray(lambda_r, np.float64).reshape(N, K)
    spack[:, 16] = np.asarray(sigma2, np.float64)
    spack[PRE:, 17] = 1.0

    base = {"bdpack": f32(bdpack), "spack": f32(spack)}

    in_maps = []
    for c in range(NCORES):
        t0 = c * TC
        yc = np.zeros((TW, N), np.float32)
        if c > 0:
            yc[0:PRE, :] = Y[t0 - PRE:t0]
        yc[PRE:, :] = Y[t0:t0 + TC]

        kpack = np.zeros((K, 32), np.float32)
        rpk = np.zeros((1, 1172), np.float64)
        rpk[0, 0:1024] = hc["info_row"]
        rpk[0, 1024:1152] = np.asarray(sigma2, np.float64)
        rpk[0, 1152:1168] = np.asarray(mu, np.float64)
        epk = np.zeros((NE, 1024), np.float64)
        if c == 0:
            kpack[:, 0:8] = hc["seq8"]
            kpack[:, 8:16] = 1.0      # em8
            kpack[:, 24 + NE - 1] = 1.0  # e7 col 7
            epk[:, :] = hc["info_early"]
            rpk[0, 1168] = hc["ld_early"] + (TC - NE) * hc["ld_ss"]
            rpk[0, 1169] = hc["corr"]
        else:
            kpack[:, 16:24] = 1.0     # oem8
            epk[:, :] = hc["info_row"][None, :]
            rpk[0, 1168] = TC * hc["ld_ss"]

        m = dict(base)
        m["y_chunk"] = yc
        m["kpack"] = kpack
        m["rpack"] = f32(rpk)
        m["epack"] = f32(epk)
        in_maps.append(m)

    if "nc" not in _CACHED:
        _CACHED["nc"] = _build_nc()
    res = run_bass_kernel_spmd(_CACHED["nc"], in_maps, list(range(NCORES)),
                               trace=_trace)
    _CACHED["last_result"] = res

    means = np.concatenate([res.results[c]["means_part"] for c in range(NCORES)], axis=0)
    infos = np.concatenate([res.results[c]["infos_part"] for c in range(NCORES)],
                           axis=0).reshape(T, 2 * K, 2 * K)
    ll = np.float32(np.sum([res.results[c]["ll_part"][0, 0] for c in range(NCORES)],
                           dtype=np.float64))
    return means, infos, ll


# revision 21
# speedup vs baseline: 1.6213x; 1.2573x over previous
"""Bellman/Kalman information filter (DFSV model) on 8 Trainium2 NeuronCores.

Structure exploited (all verified against the reference):
  - F and Q are block-diagonal and H = [Lambda, 0], so the information
    matrix stays exactly block-diagonal forever and the h-mean is never
    updated: h_t == mu for all t, and the h covariance block sits at its
    stationary Lyapunov solution, so Omega_h is constant.
  - The f-block covariance Riccati recursion is data-independent (Q_f =
    diag(exp(mu)) is constant) and converges (bitwise in fp32) by t=8.
  - Past convergence the mean recursion is linear time-invariant:
        x_t = A x_{t-1} + P_post * g_t,   A = (I - P_post J) Phi_f
    with spectral radius ~0.27, so a 16-tap truncated FIR reproduces the
    scan to < 1e-9 relative error.  Each core therefore computes its
    1024-step chunk independently from a 32-step warm-up window.
  - Log-lik terms via Woodbury: logdet S_t = sum(log sigma2) + ld(t) with
    ld(t) = logdet(I + J P_pred(t)) (data-independent, constant for t>=8)
    and v' S^-1 v = y'R^-1 y - f_pred'(g+u) - u' P_post u  (all K-dim).

Per-core device kernel does all O(T)-work: the Y pass (transpose + big
matmuls), the blocked-partition FIR scan (8 time-groups of 128 steps on
the partition axis, log2 doubling with block-diagonal A-power matmuls),
the innovation quadratics (accumulated into a single PSUM scalar), and
the means/infos tensor assembly + writeback (33 MB total).  Host
precomputes only the tiny (16x16, data-independent) Riccati / Lyapunov
constants and the 8-step exact prefix, shards the inputs, and
concatenates the outputs.
"""

import numpy as np

import concourse.bass as bass
import concourse.bacc as bacc
import concourse.mybir as mybir
from concourse import tile
from concourse.bass_utils import run_bass_kernel_spmd
from concourse.masks import make_identity

F32 = mybir.dt.float32
T, N, K = 8192, 128, 16
NCORES = 8
TC = T // NCORES          # 1024 steps per core
PRE = 32                  # warm-up rows prepended to each chunk
TW = TC + PRE             # 1056
NGRP = 8                  # time-groups per core (partition-blocked FIR)
GW = 144                  # group width: 16 warm-up cols + 128 steps
VO = 16                   # first valid column within a group
NE = 8                    # exact early steps (Riccati converged after)

_CACHED = {}


def _bd8(m):
    """Block-diagonal [128,128] with 8 copies of a 16x16 block."""
    out = np.zeros((128, 128), np.float64)
    for g in range(8):
        out[16 * g:16 * g + 16, 16 * g:16 * g + 16] = m
    return out


def _build_nc():
    nc = bacc.Bacc(None, target_bir_lowering=False)

    y_in = nc.dram_tensor("y_chunk", [TW, N], F32, kind="ExternalInput")
    # bdpack columns: [0:128] Phi', [128:256] J, [256:384] P_ss,
    #                 [384:896] (A^1)', (A^2)', (A^4)', (A^8)'
    bdpack = nc.dram_tensor("bdpack", [128, 896], F32, kind="ExternalInput")
    # spack columns: [0:16] lambda, [16] sigma2, [17] warm-up row mask
    spack = nc.dram_tensor("spack", [128, 18], F32, kind="ExternalInput")
    # kpack columns: [0:8] seq8, [8:16] em8, [16:24] oem8, [24:32] e7
    kpack = nc.dram_tensor("kpack", [K, 32], F32, kind="ExternalInput")
    # rpack (single row, base partition 0): [0:1024] info_row,
    #   [1024:1152] sigma2 as a row, [1152:1168] mu, [1168:1172] consts
    rpack = nc.dram_tensor("rpack", [1, 1172], F32, kind="ExternalInput")
    # epack: the 8 early info rows (DMA'd straight to the output)
    epack = nc.dram_tensor("epack", [NE, 1024], F32, kind="ExternalInput")
    means_o = nc.dram_tensor("means_part", [TC, 2 * K], F32, kind="ExternalOutput")
    infos_o = nc.dram_tensor("infos_part", [TC, 1024], F32, kind="ExternalOutput")
    ll_o = nc.dram_tensor("ll_part", [1, 1], F32, kind="ExternalOutput")

    LOG2PI = float(np.log(2.0 * np.pi))

    with tile.TileContext(nc) as tc:
        from contextlib import ExitStack
        with ExitStack() as ctx:
            cst = ctx.enter_context(tc.tile_pool(name="cst", bufs=1))
            wrk = ctx.enter_context(tc.tile_pool(name="wrk", bufs=2))
            yin_pool = ctx.enter_context(tc.tile_pool(name="yinp", bufs=4))
            ps = ctx.enter_context(tc.tile_pool(name="ps", bufs=3, space="PSUM"))
            psg = ctx.enter_context(tc.tile_pool(name="psg", bufs=3, space="PSUM"))
            pscal = ctx.enter_context(tc.tile_pool(name="pscal", bufs=1, space="PSUM"))

            # ---------- input DMAs (y on sync queue, packs on scalar queue) ----
            ytiles = []
            nblk = TW // 128            # 8 full blocks + 32-row remainder
            rem = TW - 128 * nblk       # 32
            for i in range(nblk + 1):
                yt = yin_pool.tile([128, 128], F32, tag="yin")
                r1_ = min(TW, 128 * (i + 1))
                nc.sync.dma_start(out=yt[0:r1_ - 128 * i, :],
                                  in_=y_in[128 * i:r1_, :])
                ytiles.append(yt)

            bd_sb = cst.tile([128, 896], F32)
            nc.scalar.dma_start(out=bd_sb[:], in_=bdpack[:])
            bd_phi = bd_sb[:, 0:128]
            bd_j = bd_sb[:, 128:256]
            bd_pp = bd_sb[:, 256:384]
            bdp = [bd_sb[:, 384 + 128 * k:512 + 128 * k] for k in range(4)]
            sp_sb = cst.tile([128, 18], F32)
            nc.scalar.dma_start(out=sp_sb[:], in_=spack[:])
            lam_sb = sp_sb[:, 0:16]
            sig_sb = sp_sb[:, 16:17]
            mask0 = sp_sb[:, 17:18]
            kp_sb = cst.tile([K, 32], F32)
            nc.scalar.dma_start(out=kp_sb[:], in_=kpack[:])
            seq8, em8, oem8, e7 = (kp_sb[:, 8 * j:8 * (j + 1)] for j in range(4))
            rp_sb = cst.tile([1, 1172], F32)
            nc.scalar.dma_start(out=rp_sb[:], in_=rpack[:])
            ep_sb = cst.tile([NE, 1024], F32)
            nc.scalar.dma_start(out=ep_sb[:], in_=epack[:])
            irow = rp_sb[:, 0:1024]
            sig_row = rp_sb[:, 1024:1152]
            mu_sb = rp_sb[:, 1152:1168]
            consts = rp_sb[:, 1168:1172]

            # ---------- tiny constants ----------
            ident = cst.tile([128, 128], F32)
            make_identity(nc, ident[:])
            ones_row = cst.tile([1, 128], F32)
            nc.gpsimd.memset(ones_row[:], 1.0)
            neg128 = cst.tile([128, 1], F32)
            nc.gpsimd.memset(neg128[:], -1.0)
            n1024 = cst.tile([128, 1], F32)
            nc.gpsimd.memset(n1024[:], float(TC))

            isig = cst.tile([128, 1], F32)
            nc.vector.reciprocal(isig[:], sig_sb)
            ltri = cst.tile([N, K], F32)
            nc.vector.tensor_scalar_mul(ltri[:], lam_sb, isig[:])

            # ---------- Y pass: all transposes, then chunked G ----------
            yT = wrk.tile([128, TW], F32, tag="yT")
            for i in range(nblk + 1):
                yt = ytiles[i]
                cw = 128 if i < nblk else rem
                pt = ps.tile([128, 128], F32, tag="big")
                nc.tensor.transpose(pt[:, 0:cw], yt[0:cw, :], ident[0:cw, 0:cw])
                nc.vector.tensor_copy(yT[:, 128 * i:128 * i + cw], pt[:, 0:cw])
            g_sb = wrk.tile([K, TW], F32, tag="g")
            for c0, cw in ((0, 512), (512, 512), (1024, TW - 1024)):
                pgb = psg.tile([K, 512], F32, tag="gband")
                nc.tensor.matmul(pgb[:, 0:cw], lhsT=ltri[:], rhs=yT[:, c0:c0 + cw],
                                 start=True, stop=True)
                nc.vector.tensor_copy(g_sb[:, c0:c0 + cw], pgb[:, 0:cw])
            # y'R^-1 y: square in yT layout (GpSimd), reduce valid cols (DVE)
            ysqT = wrk.tile([128, TW], F32, tag="ysqT")
            for c0, cw in ((0, 512), (512, 512), (1024, TW - 1024)):
                nc.gpsimd.tensor_mul(ysqT[:, c0:c0 + cw], yT[:, c0:c0 + cw],
                                     yT[:, c0:c0 + cw])
            rysq = wrk.tile([128, 1], F32, tag="rysq")
            nc.vector.tensor_reduce(rysq[:], ysqT[:, PRE:TW],
                                    axis=mybir.AxisListType.X,
                                    op=mybir.AluOpType.add)

            # ---------- infos output (overlaps the FIR chain) ----------
            binfo = cst.tile([128, 1024], F32)
            for h in range(2):
                pb = ps.tile([128, 512], F32, tag="big")
                nc.tensor.matmul(pb[:], lhsT=ones_row[:],
                                 rhs=irow[:, 512 * h:512 * (h + 1)],
                                 start=True, stop=True)
                nc.vector.tensor_copy(binfo[:, 512 * h:512 * (h + 1)], pb[:])
            nc.sync.dma_start(out=infos_o[0:NE, :], in_=ep_sb[:])
            for i in range(7):
                nc.sync.dma_start(out=infos_o[NE + 128 * i:NE + 128 * (i + 1), :],
                                  in_=binfo[:])
            nc.sync.dma_start(out=infos_o[NE + 128 * 7:TC, :],
                              in_=binfo[0:TC - NE - 128 * 7, :])

            # ---------- blocked layout: 8 time-groups on partitions ----------
            gblk = wrk.tile([128, GW], F32, tag="gblk")
            for g in range(NGRP):
                og = PRE + 128 * g - VO
                nc.sync.dma_start(out=gblk[16 * g:16 * (g + 1), :],
                                  in_=g_sb[:, og:og + GW])

            pbb = ps.tile([128, GW], F32, tag="big")
            nc.tensor.matmul(pbb[:], lhsT=bd_pp, rhs=gblk[:], start=True, stop=True)
            bblk = wrk.tile([128, GW], F32, tag="xfir")
            nc.vector.tensor_copy(bblk[:], pbb[:])

            # c-surgery: zero the pre-convergence prefix, inject exact x_7
            # (data-driven; masks make it a no-op on cores > 0)
            t1 = wrk.tile([K, NE], F32, tag="t1")
            t2 = wrk.tile([K, NE], F32, tag="t2")
            nc.vector.tensor_mul(t1[:], bblk[0:16, VO:VO + NE], oem8)
            nc.vector.tensor_mul(t2[:], seq8, e7)
            nc.vector.tensor_add(bblk[0:16, VO:VO + NE], t1[:], t2[:])

            # ---------- FIR scan via log2 doubling ----------
            xk = bblk
            for k in range(4):
                s = 1 << k
                px = ps.tile([128, GW], F32, tag="big")
                nc.tensor.matmul(px[:, s:GW], lhsT=bdp[k], rhs=xk[:, 0:GW - s],
                                 start=True, stop=True)
                xk2 = wrk.tile([128, GW], F32, tag="xfir")
                nc.vector.tensor_copy(xk2[:, 0:s], xk[:, 0:s])
                nc.vector.tensor_add(xk2[:, s:GW], px[:, s:GW], xk[:, s:GW])
                xk = xk2

            # blend exact early means into group 0 (no-op on cores > 0)
            b1 = wrk.tile([K, NE], F32, tag="t1")
            b2 = wrk.tile([K, NE], F32, tag="t2")
            nc.vector.tensor_mul(b1[:], xk[0:16, VO:VO + NE], oem8)
            nc.vector.tensor_mul(b2[:], seq8, em8)
            nc.vector.tensor_add(xk[0:16, VO:VO + NE], b1[:], b2[:])

            # ---------- F_pred / U / W ----------
            pf = ps.tile([128, GW], F32, tag="big")
            nc.tensor.matmul(pf[:, VO:GW], lhsT=bd_phi, rhs=xk[:, VO - 1:GW - 1],
                             start=True, stop=True)
            fp = wrk.tile([128, GW], F32, tag="fp")
            nc.vector.tensor_copy(fp[:, VO:GW], pf[:, VO:GW])

            pu = ps.tile([128, GW], F32, tag="big")
            nc.tensor.matmul(pu[:, VO:GW], lhsT=bd_j, rhs=fp[:, VO:GW],
                             start=True, stop=True)
            ub = wrk.tile([128, GW], F32, tag="ub")
            nc.vector.tensor_sub(ub[:, VO:GW], gblk[:, VO:GW], pu[:, VO:GW])

            pw = ps.tile([128, GW], F32, tag="big")
            nc.tensor.matmul(pw[:, VO:GW], lhsT=bd_pp, rhs=ub[:, VO:GW],
                             start=True, stop=True)
            wb = wrk.tile([128, 128], F32, tag="wb")
            nc.vector.tensor_copy(wb[:], pw[:, VO:GW])

            s1 = wrk.tile([128, 128], F32, tag="s1")
            nc.vector.tensor_add(s1[:], gblk[:, VO:GW], ub[:, VO:GW])
            prod1 = wrk.tile([128, 128], F32, tag="prod1")
            nc.vector.tensor_mul(prod1[:], fp[:, VO:GW], s1[:])
            prod2 = wrk.tile([128, 128], F32, tag="prod2")
            nc.vector.tensor_mul(prod2[:], ub[:, VO:GW], wb[:])
            r1 = wrk.tile([128, 1], F32, tag="r1")
            nc.vector.tensor_reduce(r1[:], prod1[:], axis=mybir.AxisListType.X,
                                    op=mybir.AluOpType.add)
            r2 = wrk.tile([128, 1], F32, tag="r2")
            nc.vector.tensor_reduce(r2[:], prod2[:], axis=mybir.AxisListType.X,
                                    op=mybir.AluOpType.add)

            # ---------- log-lik scalar ----------
            logsig = cst.tile([128, 1], F32)
            nc.scalar.activation(logsig[:], sig_sb, mybir.ActivationFunctionType.Ln)
            pscal_t = pscal.tile([1, 1], F32)
            nc.tensor.matmul(pscal_t[:], lhsT=isig[:], rhs=rysq[:], start=True, stop=False)
            nc.tensor.matmul(pscal_t[:], lhsT=neg128[:], rhs=r1[:], start=False, stop=False)
            nc.tensor.matmul(pscal_t[:], lhsT=neg128[:], rhs=r2[:], start=False, stop=False)
            nc.tensor.matmul(pscal_t[:], lhsT=n1024[:], rhs=logsig[:], start=False, stop=True)
            lt1 = wrk.tile([1, 1], F32, tag="lt1")
            nc.vector.tensor_add(lt1[:], pscal_t[:], consts[:, 0:1])
            lt2 = wrk.tile([1, 1], F32, tag="lt2")
            nc.vector.tensor_add(lt2[:], lt1[:], consts[:, 1:2])
            ll_sb = wrk.tile([1, 1], F32, tag="llsb")
            nc.scalar.activation(ll_sb[:], lt2[:], mybir.ActivationFunctionType.Copy,
                                 scale=-0.5, bias=-0.5 * TC * N * LOG2PI)
            nc.sync.dma_start(out=ll_o[:], in_=ll_sb[:])

            # ---------- means output ----------
            pmu = ps.tile([128, 32], F32, tag="big")
            nc.tensor.matmul(pmu[:, 0:16], lhsT=ones_row[:], rhs=mu_sb,
                             start=True, stop=True)
            mub = wrk.tile([128, 16], F32, tag="mub")
            nc.vector.tensor_copy(mub[:], pmu[:, 0:16])
            pxt = ps.tile([128, 128], F32, tag="big")
            nc.tensor.transpose(pxt[:], xk[:, VO:VO + 128], ident[:])
            mtall = wrk.tile([128, 256], F32, tag="mtall")
            for g in range(NGRP):
                nc.vector.tensor_copy(mtall[:, 32 * g:32 * g + 16],
                                      pxt[:, 16 * g:16 * (g + 1)])
                nc.vector.tensor_copy(mtall[:, 32 * g + 16:32 * g + 32], mub[:])
            for g in range(NGRP):
                eng = nc.scalar if g % 2 else nc.sync
                eng.dma_start(out=means_o[128 * g:128 * (g + 1), :],
                              in_=mtall[:, 32 * g:32 * (g + 1)])

    nc.finalize()
    return nc


def _host_constants(lam, Phi_f, Phi_h, mu, sigma2, Q_h, Y):
    """Tiny data-independent constants (float64) + the 8-step exact prefix."""
    lam = lam.astype(np.float64)
    Phi_f = Phi_f.astype(np.float64)
    Phi_h = Phi_h.astype(np.float64)
    mu = mu.astype(np.float64)
    sigma2 = sigma2.astype(np.float64)
    Q_h = Q_h.astype(np.float64)
    I = np.eye(K)

    LtRi = lam.T / sigma2
    J = LtRi @ lam
    Qf = np.diag(np.exp(mu))

    # h block: stationary Lyapunov solution
    P_h = np.linalg.solve(np.eye(K * K) - np.kron(Phi_h, Phi_h), Q_h.reshape(-1)).reshape(K, K)
    P_h = 0.5 * (P_h + P_h.T)
    Om_h = np.linalg.inv(P_h)

    # f block Riccati (data independent); converged long before t=16
    P_post = I.copy()
    Ppost_l, Om_l, ld_l = [], [], []
    for _ in range(16):
        P_pred = Phi_f @ P_post @ Phi_f.T + Qf
        Om_pred = np.linalg.inv(P_pred)
        Om_post = Om_pred + J
        P_post = np.linalg.inv(Om_post)
        P_post = 0.5 * (P_post + P_post.T)
        Ppost_l.append(P_post)
        Om_l.append(Om_post)
        ld_l.append(np.linalg.slogdet(I + J @ P_pred)[1])
    P_ss = Ppost_l[-1]
    A_ss = (I - P_ss @ J) @ Phi_f
    Om_ss = Om_l[-1]
    ld_ss = ld_l[-1]

    pows = [A_ss]
    for _ in range(3):
        pows.append(pows[-1] @ pows[-1])

    # exact 8-step prefix on core 0's data
    x = np.zeros(K)
    seq8 = np.zeros((K, NE))
    corr = 0.0
    for t_ in range(NE):
        fpred = Phi_f @ x
        g = LtRi @ Y[t_].astype(np.float64)
        u = g - J @ fpred
        x = fpred + Ppost_l[t_] @ u
        seq8[:, t_] = x
        corr += u @ ((P_ss - Ppost_l[t_]) @ u)

    def info_flat(om_f):
        row = np.zeros(1024)
        for k_ in range(K):
            row[32 * k_:32 * k_ + K] = om_f[k_]
            row[32 * (K + k_) + K:32 * (K + k_) + 2 * K] = Om_h[k_]
        return row

    info_row = info_flat(Om_ss)
    info_early = np.stack([info_flat(Om_l[t_]) for t_ in range(NE)])

    return dict(
        J=J, P_ss=P_ss, A_pows=pows, Phi_f=Phi_f,
        seq8=seq8, corr=corr, info_row=info_row, info_early=info_early,
        ld_early=float(np.sum(ld_l[:NE])), ld_ss=ld_ss,
    )


def kernel(observations, lambda_r, Phi_f, Phi_h, mu, sigma2, Q_h, _trace=False):
    Y = np.ascontiguousarray(observations, np.float32)
    hc = _host_constants(lambda_r, Phi_f, Phi_h, mu, sigma2, Q_h, Y)

    f32 = lambda a: np.ascontiguousarray(a, np.float32)

    bdpack = np.zeros((128, 896), np.float64)
    bdpack[:, 0:128] = _bd8(hc["Phi_f"].T)
    bdpack[:, 128:256] = _bd8(hc["J"])
    bdpack[:, 256:384] = _bd8(hc["P_ss"])
    for k in range(4):
        bdpack[:, 384 + 128 * k:512 + 128 * k] = _bd8(hc["A_pows"][k].T)

    spack = np.zeros((128, 18), np.float64)
    spack[:, 0:16] = np.asarray(lambda_r, np.float64).reshape(N, K)
    spack[:, 16] = np.asarray(sigma2, np.float64)
    spack[PRE:, 17] = 1.0

    base = {"bdpack": f32(bdpack), "spack": f32(spack)}

    in_maps = []
    for c in range(NCORES):
        t0 = c * TC
        yc = np.zeros((TW, N), np.float32)
        if c > 0:
            yc[0:PRE, :] = Y[t0 - PRE:t0]
        yc[PRE:, :] = Y[t0:t0 + TC]

        kpack = np.zeros((K, 32), np.float32)
        rpk = np.zeros((1, 1172), np.float64)
        rpk[0, 0:1024] = hc["info_row"]
        rpk[0, 1024:1152] = np.asarray(sigma2, np.float64)
        rpk[0, 1152:1168] = np.asarray(mu, np.float64)
        epk = np.zeros((NE, 1024), np.float64)
        if c == 0:
            kpack[:, 0:8] = hc["seq8"]
            kpack[:, 8:16] = 1.0      # em8
            kpack[:, 24 + NE - 1] = 1.0  # e7 col 7
            epk[:, :] = hc["info_early"]
            rpk[0, 1168] = hc["ld_early"] + (TC - NE) * hc["ld_ss"]
            rpk[0, 1169] = hc["corr"]
        else:
            kpack[:, 16:24] = 1.0     # oem8
            epk[:, :] = hc["info_row"][None, :]
            rpk[0, 1168] = TC * hc["ld_ss"]

        m = dict(base)
        m["y_chunk"] = yc
        m["kpack"] = kpack
        m["rpack"] = f32(rpk)
        m["epack"] = f32(epk)
        in_maps.append(m)

    if "nc" not in _CACHED:
        _CACHED["nc"] = _build_nc()
    res = run_bass_kernel_spmd(_CACHED["nc"], in_maps, list(range(NCORES)),
                               trace=_trace)
    _CACHED["last_result"] = res

    means = np.concatenate([res.results[c]["means_part"] for c in range(NCORES)], axis=0)
    infos = np.concatenate([res.results[c]["infos_part"] for c in range(NCORES)],
                           axis=0).reshape(T, 2 * K, 2 * K)
    ll = np.float32(np.sum([res.results[c]["ll_part"][0, 0] for c in range(NCORES)],
                           dtype=np.float64))
    return means, infos, ll
